# revision 23
# baseline (speedup 1.0000x reference)
"""Chamfer distance (adv->ori direction) Trainium2 Bass kernel, v20.

Problem: adv_pc [8, 4096, 3], ori_pc [8, 4096, 3], weights [8] ->
scalar f32 loss = mean_b( w_b * mean_k( min_j ||adv_bk - ori_bj||^2 ) ).

Sharding: data parallel over the batch dim - core b handles batch b.

v15 is a sorted-window kNN with a provably-sufficient dense fallback,
replacing the dense 4096x4096 scan (which is consumption-bound: DVE +
ScalarE drain PSUM at ~2 elem/ns/lane, a ~90us floor).

Algorithm (per batch):
  * HOST: sort both point sets by x. For the 128 rank-consecutive adv
    points of tile t, pass 1 on device scans only the ori rank-window
    [s_t, s_t+W), W = 384, s_t = clamp(128t-128, 0, 4096-W)
    (10.7x fewer distances than dense).
  * HOST flags points whose window min cannot be PROVEN exact: points
    outside the window satisfy d >= gap^2 where gap is the x-distance
    from the query to the window edge (x-sorted). An upper bound u2 on
    the true NN distance comes from 3x(+-64) rank-local candidates in
    x-, y-, and z-sorted orders. u2 < gap^2 => window min IS the true
    min. Measured flags on the actual data: <= 99 per batch; fallback
    capacity is 128 (overflow handled by extra device launches).
  * Pass 2 on device: full 4096-j dense scan for one tile of gathered
    flagged points (padded).
  * Device returns raw per-point min(m') (m' = b2/2 - a.b); host
    combines (d = a2 + 2*min m', in f64), patches flagged points with
    their pass-2 mins, and takes the weighted mean.

All matmul operands are host-prepped (bf16 hi/lo split, coordinate-
major contract-12 layout [1,-ax,-ay,-az] x [b2/2,bx,by,bz] with the
exact 3-term decomposition Ah.Bh+Ah.Bl+Al.Bh), so the device does no
staging beyond 3 input DMAs + row-group replica DMAs for the 4-way
PE quadrant concurrency.

Device pass 1: 8 super-waves of 4 window-tiles in one [128, 2048]
PSUM tile. Each tile's whole W-col window is ONE matmul into its own
PSUM bank on its own PE quadrant (two quadrants writing the same bank
hangs the device - learned the hard way), and ONE fused tensor_reduce
over the (4, W-of-512) bank view emits all 4 tile minima.
Measured: 37632 ns vs the 142110 ns dense v12 baseline (3.78x),
rel err 4.4e-4. Remaining: ~11us fixed startup (framework preamble +
operand landing), ~19us of back-to-back DVE tensor_reduce (the PSUM
drain floor for 32x384 window cols at 1 elem/cycle/lane), ~2.8us of
replica/chunk-arrival bubbles at waves 2-3, ~4us drain tail.
Everything stays exact f32: direct PSUM reduce (1 elem/cycle)
measured faster than any ScalarE-convert path (tensor_scalar
accum-min runs ~1x on HW with a separate accumulator-readout op, not
the 4x the cost model promises).
"""

import numpy as np

B = 8
K = 4096
KT = K // 128   # 32 window tiles
W = 384         # ori window width per tile
M = 128         # left margin in ranks
CAND = 64       # host candidate half-width per sort axis
FB_TILES = 1    # fallback capacity = 128 * FB_TILES (28 max measured)
NCORES = 8

_NC_CACHE = {}

# static window starts, shared by device builder and host driver
S_T = [min(max(128 * t - M, 0), K - W) for t in range(KT)]


def _build_nc():
    import concourse.bacc as bacc
    import concourse.mybir as mybir
    import concourse.tile as tile

    f32 = mybir.dt.float32
    bf16 = mybir.dt.bfloat16
    Alu = mybir.AluOpType
    Ax = mybir.AxisListType

    nc = bacc.Bacc("TRN2", target_bir_lowering=False, debug=False,
                   num_devices=NCORES)

    hla = nc.dram_tensor("hla", [12, K], bf16, kind="ExternalInput").ap()
    hlo = nc.dram_tensor("hlo", [12, K], bf16, kind="ExternalInput").ap()
    hlf = nc.dram_tensor("hlf", [12, 128 * FB_TILES], bf16,
                         kind="ExternalInput").ap()
    out = nc.dram_tensor("out", [128, KT + 2 * FB_TILES], f32,
                         kind="ExternalOutput").ap()

    with tile.TileContext(nc) as tc:
        with tc.tile_pool(name="sb", bufs=1) as sb:
            HLa = sb.tile([128, K], bf16)
            HLo = sb.tile([128, K], bf16)
            HLf = sb.tile([128, 128 * FB_TILES], bf16)
            g = sb.tile([128, KT + 2 * FB_TILES], f32)

            # land operands in 2 column-chunks and replicate rows 0-11
            # to PE row groups 32/64/96 per chunk, so wave 0 (needs
            # only chunk-1 rows 0:12) and wave 1+ (chunk-1 replicas)
            # start ~2us before the full tensors land. The gpsimd SWDGE
            # queue stays completely unused - its drains are expensive.
            # Queue order: both landing chunks FIRST, then the chunk-1
            # replicas (needed by wave 2), then chunk-2 replicas (wave
            # 3+) - replicas queued behind a land chunk were arriving
            # ~2us after the waves that needed them.
            HK = K // 2
            nc.sync.dma_start(out=HLo[0:12, 0:HK], in_=hlo[:, 0:HK])
            nc.scalar.dma_start(out=HLa[0:12, 0:HK], in_=hla[:, 0:HK])
            nc.sync.dma_start(out=HLo[0:12, HK:K], in_=hlo[:, HK:K])
            nc.scalar.dma_start(out=HLa[0:12, HK:K], in_=hla[:, HK:K])
            for r in (32, 64, 96):
                nc.sync.dma_start(out=HLo[r:r + 12, 0:HK],
                                  in_=HLo[0:12, 0:HK])
                nc.scalar.dma_start(out=HLa[r:r + 12, 0:HK],
                                    in_=HLa[0:12, 0:HK])
            for r in (32, 64, 96):
                nc.sync.dma_start(out=HLo[r:r + 12, HK:K],
                                  in_=HLo[0:12, HK:K])
                nc.scalar.dma_start(out=HLa[r:r + 12, HK:K],
                                    in_=HLa[0:12, HK:K])
            nc.scalar.dma_start(out=HLf[0:12, :], in_=hlf)
            for r in (32, 64, 96):
                nc.sync.dma_start(out=HLf[r:r + 12, :], in_=HLf[0:12, :])

            # ---- pass 1: 8 super-waves of 4 window-tiles each ----
            with tc.tile_pool(name="mm1", bufs=2, space="PSUM") as mm1:
                for v in range(KT // 4):
                    ps = mm1.tile([128, 2048], f32, tag="ps")
                    for q in range(4):
                        t = 4 * v + q
                        # EARLY: the first two super-waves fill on
                        # quadrant 0 only (serial matmuls) - rows 0:12
                        # arrive ~2us before the quadrant replicas, and
                        # wave 1's serial fill hides the replica wait.
                        r = 0 if v < 2 else 32 * q
                        nc.tensor.matmul(
                            ps[:, q * 512:q * 512 + W],
                            HLa[r:r + 12, t * 128:(t + 1) * 128],
                            HLo[r:r + 12, S_T[t]:S_T[t] + W],
                            start=True, stop=True, tile_position=(r, 0))
                    ps_b = ps[:].rearrange("p (x c) -> p x c", c=512)
                    nc.vector.tensor_reduce(
                        g[:, 4 * v:4 * v + 4], ps_b[:, :, 0:W],
                        axis=Ax.X, op=Alu.min)

                # ---- pass 2: dense full scan for FB_TILES gathered
                # tiles, 2 waves of [128, 2048], one fused reduce each
                # (same pool: no extra pool-close barrier).
                for w in range(2 * FB_TILES):
                    ft, h = divmod(w, 2)
                    ps = mm1.tile([128, 2048], f32, tag="ps")
                    for q in range(4):
                        r = 32 * q
                        nc.tensor.matmul(
                            ps[:, q * 512:(q + 1) * 512],
                            HLf[r:r + 12, ft * 128:(ft + 1) * 128],
                            HLo[r:r + 12,
                                h * 2048 + q * 512:h * 2048 + (q + 1) * 512],
                            start=True, stop=True, tile_position=(r, 0))
                    nc.vector.tensor_reduce(
                        g[:, KT + w:KT + w + 1], ps[:], axis=Ax.X,
                        op=Alu.min)

            nc.sync.dma_start(out=out, in_=g[:])

    nc.compile()
    return nc


def _get_nc():
    if "nc" not in _NC_CACHE:
        _NC_CACHE["nc"] = _build_nc()
    return _NC_CACHE["nc"]


def _bf16(x):
    """round-to-nearest-even f32 -> bf16, kept as f32 values."""
    u = x.astype(np.float32).view(np.uint32)
    rounded = (u + 0x7FFF + ((u >> 16) & 1)) & 0xFFFF0000
    return rounded.view(np.float32)


def _prepare(adv_pc, ori_pc):
    """Host prep: sort, flag, build device operand in_maps + contexts."""
    import ml_dtypes

    FBC = 128 * FB_TILES
    in_maps = []
    post = []  # per-batch host context for combining
    for b in range(B):
        a = adv_pc[b]
        o = ori_pc[b]
        ia = np.argsort(a[:, 0], kind="stable")
        io = np.argsort(o[:, 0], kind="stable")
        aS = a[ia]
        oS = o[io]
        aS64 = aS.astype(np.float64)
        oS64 = oS.astype(np.float64)

        # host upper bound on NN dist^2: +-CAND rank-local candidates
        # in x-, y-, z-sorted orders.
        u2 = np.full(K, np.inf)
        arange = np.arange(K)
        for dlt in range(-CAND, CAND):
            idx = np.clip(arange + dlt, 0, K - 1)
            u2 = np.minimum(u2, ((aS64 - oS64[idx]) ** 2).sum(-1))
        for ax in (1, 2):
            ja = np.argsort(a[:, ax], kind="stable")
            jo = np.argsort(o[:, ax], kind="stable")
            ar = np.empty(K, np.int64)
            ar[ja] = arange
            aR = ar[ia]  # ax-rank of each x-sorted adv point
            oA = o[jo].astype(np.float64)
            for dlt in range(-CAND, CAND):
                idx = np.clip(aR + dlt, 0, K - 1)
                u2 = np.minimum(u2, ((aS64 - oA[idx]) ** 2).sum(-1))

        # exactness test: outside-window distance lower bound gap^2
        t_of = arange // 128
        s = np.array(S_T)[t_of]
        gl = np.where(s == 0, np.inf, aS64[:, 0] - oS64[s, 0])
        gr = np.where(s == K - W, np.inf, oS64[s + W - 1, 0] - aS64[:, 0])
        gap = np.minimum(gl, gr)
        gap2 = np.where(gap > 0, gap * gap, 0.0)
        flag = u2 >= gap2 * 0.98
        fidx = np.nonzero(flag)[0]

        # device operand layouts (bf16 hi/lo split, contract-12)
        o4 = np.empty((4, K), np.float32)
        o4[0] = (oS64 ** 2).sum(-1).astype(np.float32) * 0.5
        o4[1:] = oS.T
        a4 = np.empty((4, K), np.float32)
        a4[0] = 1.0
        a4[1:] = -aS.T
        ohi = _bf16(o4)
        olo = _bf16(o4 - ohi)
        ahi = _bf16(a4)
        alo = _bf16(a4 - ahi)
        hlo = np.concatenate([ohi, olo, ohi], 0)   # [Bh; Bl; Bh]
        hla = np.concatenate([ahi, ahi, alo], 0)   # [Ah; Ah; Al]
        f_pad = np.zeros(FBC, np.int64)
        nf = min(len(fidx), FBC)
        f_pad[:nf] = fidx[:nf]
        hlf = hla[:, f_pad]

        in_maps.append({
            "hla": hla.astype(ml_dtypes.bfloat16),
            "hlo": hlo.astype(ml_dtypes.bfloat16),
            "hlf": np.ascontiguousarray(hlf).astype(ml_dtypes.bfloat16),
        })
        post.append((ia, aS64, fidx, f_pad))
    return in_maps, post


def _fb_mins(gv):
    """[128, KT+2*FB] device output -> flat [128*FB] fallback mins."""
    gf = gv[:, KT:].reshape(128, FB_TILES, 2)
    fmin = np.minimum(gf[:, :, 0], gf[:, :, 1])  # [128, FB_TILES]
    return fmin.T.reshape(128 * FB_TILES)  # idx i = tile i//128, part i%128


def kernel(adv_pc, ori_pc, weights):
    from concourse.bass_utils import run_bass_kernel_spmd

    adv_pc = np.asarray(adv_pc, dtype=np.float32)
    ori_pc = np.asarray(ori_pc, dtype=np.float32)
    weights = np.asarray(weights, dtype=np.float32)

    nc = _get_nc()
    FBC = 128 * FB_TILES
    in_maps, post = _prepare(adv_pc, ori_pc)

    res = run_bass_kernel_spmd(nc, in_maps, core_ids=list(range(NCORES)))

    loss1 = np.empty(B, np.float64)
    extra = {}
    for b in range(B):
        ia, aS64, fidx, f_pad = post[b]
        gv = np.asarray(res.results[b]["out"], np.float64)
        m = gv[:, :KT].T.reshape(K)  # rank r = 128t+p -> wmin[t, p]
        fmin = _fb_mins(gv)
        nf = min(len(fidx), FBC)
        m[fidx[:nf]] = fmin[:nf]
        if len(fidx) > FBC:
            extra[b] = fidx[FBC:]
        a2 = (aS64 ** 2).sum(-1)
        loss1[b] = (a2 + 2.0 * m).mean()

    # overflow path (never hit on sane data): extra launches that
    # full-scan the remaining flagged points, FBC per launch.
    while extra:
        todo = {}
        maps2, order, chunks = [], [], {}
        for b, rest in extra.items():
            f_pad = np.zeros(FBC, np.int64)
            nf = min(len(rest), FBC)
            f_pad[:nf] = rest[:nf]
            maps2.append({
                "hla": in_maps[b]["hla"],
                "hlo": in_maps[b]["hlo"],
                "hlf": np.ascontiguousarray(
                    np.asarray(in_maps[b]["hla"])[:, f_pad]),
            })
            order.append(b)
            chunks[b] = (rest[:nf], nf)
            if len(rest) > nf:
                todo[b] = rest[nf:]
        res2 = run_bass_kernel_spmd(nc, maps2,
                                    core_ids=list(range(len(maps2))))
        for i, b in enumerate(order):
            ia, aS64, fidx, _ = post[b]
            gv2 = np.asarray(res2.results[i]["out"], np.float64)
            fmin = _fb_mins(gv2)
            rest, nf = chunks[b]
            gv = np.asarray(res.results[b]["out"], np.float64)
            mw = gv[:, :KT].T.reshape(K)
            delta = (fmin[:nf] - mw[rest]) * 2.0 / K
            loss1[b] += delta.sum()
        extra = todo

    loss = float((loss1 * weights.astype(np.float64)).mean())
    return np.array(loss, dtype=np.float32)


if __name__ == "__main__":
    rng = np.random.default_rng(0)
    a = rng.standard_normal((B, K, 3), dtype=np.float32)
    o = rng.standard_normal((B, K, 3), dtype=np.float32)
    w = np.ones((B,), dtype=np.float32)
    print(kernel(a, o, w))


# revision 24
# speedup vs baseline: 1.0024x; 1.0024x over previous
"""Chamfer distance (adv->ori direction) Trainium2 Bass kernel, v20.

Problem: adv_pc [8, 4096, 3], ori_pc [8, 4096, 3], weights [8] ->
scalar f32 loss = mean_b( w_b * mean_k( min_j ||adv_bk - ori_bj||^2 ) ).

Sharding: data parallel over the batch dim - core b handles batch b.

v15 is a sorted-window kNN with a provably-sufficient dense fallback,
replacing the dense 4096x4096 scan (which is consumption-bound: DVE +
ScalarE drain PSUM at ~2 elem/ns/lane, a ~90us floor).

Algorithm (per batch):
  * HOST: sort both point sets by x. For the 128 rank-consecutive adv
    points of tile t, pass 1 on device scans only the ori rank-window
    [s_t, s_t+W), W = 384, s_t = clamp(128t-128, 0, 4096-W)
    (10.7x fewer distances than dense).
  * HOST flags points whose window min cannot be PROVEN exact: points
    outside the window satisfy d >= gap^2 where gap is the x-distance
    from the query to the window edge (x-sorted). An upper bound u2 on
    the true NN distance comes from 3x(+-64) rank-local candidates in
    x-, y-, and z-sorted orders. u2 < gap^2 => window min IS the true
    min. Measured flags on the actual data: <= 99 per batch; fallback
    capacity is 128 (overflow handled by extra device launches).
  * Pass 2 on device: full 4096-j dense scan for one tile of gathered
    flagged points (padded).
  * Device returns raw per-point min(m') (m' = b2/2 - a.b); host
    combines (d = a2 + 2*min m', in f64), patches flagged points with
    their pass-2 mins, and takes the weighted mean.

All matmul operands are host-prepped (bf16 hi/lo split, coordinate-
major contract-12 layout [1,-ax,-ay,-az] x [b2/2,bx,by,bz] with the
exact 3-term decomposition Ah.Bh+Ah.Bl+Al.Bh), so the device does no
staging beyond 3 input DMAs + row-group replica DMAs for the 4-way
PE quadrant concurrency.

Device pass 1: 8 super-waves of 4 window-tiles in one [128, 2048]
PSUM tile. Each tile's whole W-col window is ONE matmul into its own
PSUM bank on its own PE quadrant (two quadrants writing the same bank
hangs the device - learned the hard way), and ONE fused tensor_reduce
over the (4, W-of-512) bank view emits all 4 tile minima.
Measured: 37632 ns vs the 142110 ns dense v12 baseline (3.78x),
rel err 4.4e-4 (queue-reorder variants measured equal within noise).
Everything stays exact f32: direct PSUM reduce (1 elem/cycle)
measured faster than any ScalarE-convert path (tensor_scalar
accum-min runs ~1x on HW with a separate accumulator-readout op, not
the 4x the cost model promises).
"""

import numpy as np

B = 8
K = 4096
KT = K // 128   # 32 window tiles
W = 384         # ori window width per tile
M = 128         # left margin in ranks
CAND = 64       # host candidate half-width per sort axis
FB_TILES = 1    # fallback capacity = 128 * FB_TILES (28 max measured)
NCORES = 8

_NC_CACHE = {}

# static window starts, shared by device builder and host driver
S_T = [min(max(128 * t - M, 0), K - W) for t in range(KT)]


def _build_nc():
    import concourse.bacc as bacc
    import concourse.mybir as mybir
    import concourse.tile as tile

    f32 = mybir.dt.float32
    bf16 = mybir.dt.bfloat16
    Alu = mybir.AluOpType
    Ax = mybir.AxisListType

    nc = bacc.Bacc("TRN2", target_bir_lowering=False, debug=False,
                   num_devices=NCORES)

    hla = nc.dram_tensor("hla", [12, K], bf16, kind="ExternalInput").ap()
    hlo = nc.dram_tensor("hlo", [12, K], bf16, kind="ExternalInput").ap()
    hlf = nc.dram_tensor("hlf", [12, 128 * FB_TILES], bf16,
                         kind="ExternalInput").ap()
    out = nc.dram_tensor("out", [128, KT + 2 * FB_TILES], f32,
                         kind="ExternalOutput").ap()

    with tile.TileContext(nc) as tc:
        with tc.tile_pool(name="sb", bufs=1) as sb:
            HLa = sb.tile([128, K], bf16)
            HLo = sb.tile([128, K], bf16)
            HLf = sb.tile([128, 128 * FB_TILES], bf16)
            g = sb.tile([128, KT + 2 * FB_TILES], f32)

            # land operands in 2 column-chunks and replicate rows 0-11
            # to PE row groups 32/64/96 per chunk, so wave 0 (needs
            # only chunk-1 rows 0:12) and wave 1+ (chunk-1 replicas)
            # start ~2us before the full tensors land. The gpsimd SWDGE
            # queue stays completely unused - its drains are expensive.
            HK = K // 2
            nc.sync.dma_start(out=HLo[0:12, 0:HK], in_=hlo[:, 0:HK])
            nc.scalar.dma_start(out=HLa[0:12, 0:HK], in_=hla[:, 0:HK])
            for r in (32, 64, 96):
                nc.sync.dma_start(out=HLo[r:r + 12, 0:HK],
                                  in_=HLo[0:12, 0:HK])
                nc.scalar.dma_start(out=HLa[r:r + 12, 0:HK],
                                    in_=HLa[0:12, 0:HK])
            nc.sync.dma_start(out=HLo[0:12, HK:K], in_=hlo[:, HK:K])
            nc.scalar.dma_start(out=HLa[0:12, HK:K], in_=hla[:, HK:K])
            nc.scalar.dma_start(out=HLf[0:12, :], in_=hlf)
            for r in (32, 64, 96):
                nc.sync.dma_start(out=HLo[r:r + 12, HK:K],
                                  in_=HLo[0:12, HK:K])
                nc.scalar.dma_start(out=HLa[r:r + 12, HK:K],
                                    in_=HLa[0:12, HK:K])
            for r in (32, 64, 96):
                nc.sync.dma_start(out=HLf[r:r + 12, :], in_=HLf[0:12, :])

            # ---- pass 1: 8 super-waves of 4 window-tiles each ----
            with tc.tile_pool(name="mm1", bufs=2, space="PSUM") as mm1:
                for v in range(KT // 4):
                    ps = mm1.tile([128, 2048], f32, tag="ps")
                    for q in range(4):
                        t = 4 * v + q
                        # EARLY: the first two super-waves fill on
                        # quadrant 0 only (serial matmuls) - rows 0:12
                        # arrive ~2us before the quadrant replicas, and
                        # wave 1's serial fill hides the replica wait.
                        r = 0 if v < 2 else 32 * q
                        nc.tensor.matmul(
                            ps[:, q * 512:q * 512 + W],
                            HLa[r:r + 12, t * 128:(t + 1) * 128],
                            HLo[r:r + 12, S_T[t]:S_T[t] + W],
                            start=True, stop=True, tile_position=(r, 0))
                    ps_b = ps[:].rearrange("p (x c) -> p x c", c=512)
                    nc.vector.tensor_reduce(
                        g[:, 4 * v:4 * v + 4], ps_b[:, :, 0:W],
                        axis=Ax.X, op=Alu.min)

                # ---- pass 2: dense full scan for FB_TILES gathered
                # tiles, 2 waves of [128, 2048], one fused reduce each
                # (same pool: no extra pool-close barrier).
                for w in range(2 * FB_TILES):
                    ft, h = divmod(w, 2)
                    ps = mm1.tile([128, 2048], f32, tag="ps")
                    for q in range(4):
                        r = 32 * q
                        nc.tensor.matmul(
                            ps[:, q * 512:(q + 1) * 512],
                            HLf[r:r + 12, ft * 128:(ft + 1) * 128],
                            HLo[r:r + 12,
                                h * 2048 + q * 512:h * 2048 + (q + 1) * 512],
                            start=True, stop=True, tile_position=(r, 0))
                    nc.vector.tensor_reduce(
                        g[:, KT + w:KT + w + 1], ps[:], axis=Ax.X,
                        op=Alu.min)

            nc.sync.dma_start(out=out, in_=g[:])

    nc.compile()
    return nc


def _get_nc():
    if "nc" not in _NC_CACHE:
        _NC_CACHE["nc"] = _build_nc()
    return _NC_CACHE["nc"]


def _bf16(x):
    """round-to-nearest-even f32 -> bf16, kept as f32 values."""
    u = x.astype(np.float32).view(np.uint32)
    rounded = (u + 0x7FFF + ((u >> 16) & 1)) & 0xFFFF0000
    return rounded.view(np.float32)


def _prepare(adv_pc, ori_pc):
    """Host prep: sort, flag, build device operand in_maps + contexts."""
    import ml_dtypes

    FBC = 128 * FB_TILES
    in_maps = []
    post = []  # per-batch host context for combining
    for b in range(B):
        a = adv_pc[b]
        o = ori_pc[b]
        ia = np.argsort(a[:, 0], kind="stable")
        io = np.argsort(o[:, 0], kind="stable")
        aS = a[ia]
        oS = o[io]
        aS64 = aS.astype(np.float64)
        oS64 = oS.astype(np.float64)

        # host upper bound on NN dist^2: +-CAND rank-local candidates
        # in x-, y-, z-sorted orders.
        u2 = np.full(K, np.inf)
        arange = np.arange(K)
        for dlt in range(-CAND, CAND):
            idx = np.clip(arange + dlt, 0, K - 1)
            u2 = np.minimum(u2, ((aS64 - oS64[idx]) ** 2).sum(-1))
        for ax in (1, 2):
            ja = np.argsort(a[:, ax], kind="stable")
            jo = np.argsort(o[:, ax], kind="stable")
            ar = np.empty(K, np.int64)
            ar[ja] = arange
            aR = ar[ia]  # ax-rank of each x-sorted adv point
            oA = o[jo].astype(np.float64)
            for dlt in range(-CAND, CAND):
                idx = np.clip(aR + dlt, 0, K - 1)
                u2 = np.minimum(u2, ((aS64 - oA[idx]) ** 2).sum(-1))

        # exactness test: outside-window distance lower bound gap^2
        t_of = arange // 128
        s = np.array(S_T)[t_of]
        gl = np.where(s == 0, np.inf, aS64[:, 0] - oS64[s, 0])
        gr = np.where(s == K - W, np.inf, oS64[s + W - 1, 0] - aS64[:, 0])
        gap = np.minimum(gl, gr)
        gap2 = np.where(gap > 0, gap * gap, 0.0)
        flag = u2 >= gap2 * 0.98
        fidx = np.nonzero(flag)[0]

        # device operand layouts (bf16 hi/lo split, contract-12)
        o4 = np.empty((4, K), np.float32)
        o4[0] = (oS64 ** 2).sum(-1).astype(np.float32) * 0.5
        o4[1:] = oS.T
        a4 = np.empty((4, K), np.float32)
        a4[0] = 1.0
        a4[1:] = -aS.T
        ohi = _bf16(o4)
        olo = _bf16(o4 - ohi)
        ahi = _bf16(a4)
        alo = _bf16(a4 - ahi)
        hlo = np.concatenate([ohi, olo, ohi], 0)   # [Bh; Bl; Bh]
        hla = np.concatenate([ahi, ahi, alo], 0)   # [Ah; Ah; Al]
        f_pad = np.zeros(FBC, np.int64)
        nf = min(len(fidx), FBC)
        f_pad[:nf] = fidx[:nf]
        hlf = hla[:, f_pad]

        in_maps.append({
            "hla": hla.astype(ml_dtypes.bfloat16),
            "hlo": hlo.astype(ml_dtypes.bfloat16),
            "hlf": np.ascontiguousarray(hlf).astype(ml_dtypes.bfloat16),
        })
        post.append((ia, aS64, fidx, f_pad))
    return in_maps, post


def _fb_mins(gv):
    """[128, KT+2*FB] device output -> flat [128*FB] fallback mins."""
    gf = gv[:, KT:].reshape(128, FB_TILES, 2)
    fmin = np.minimum(gf[:, :, 0], gf[:, :, 1])  # [128, FB_TILES]
    return fmin.T.reshape(128 * FB_TILES)  # idx i = tile i//128, part i%128


def kernel(adv_pc, ori_pc, weights):
    from concourse.bass_utils import run_bass_kernel_spmd

    adv_pc = np.asarray(adv_pc, dtype=np.float32)
    ori_pc = np.asarray(ori_pc, dtype=np.float32)
    weights = np.asarray(weights, dtype=np.float32)

    nc = _get_nc()
    FBC = 128 * FB_TILES
    in_maps, post = _prepare(adv_pc, ori_pc)

    res = run_bass_kernel_spmd(nc, in_maps, core_ids=list(range(NCORES)))

    loss1 = np.empty(B, np.float64)
    extra = {}
    for b in range(B):
        ia, aS64, fidx, f_pad = post[b]
        gv = np.asarray(res.results[b]["out"], np.float64)
        m = gv[:, :KT].T.reshape(K)  # rank r = 128t+p -> wmin[t, p]
        fmin = _fb_mins(gv)
        nf = min(len(fidx), FBC)
        m[fidx[:nf]] = fmin[:nf]
        if len(fidx) > FBC:
            extra[b] = fidx[FBC:]
        a2 = (aS64 ** 2).sum(-1)
        loss1[b] = (a2 + 2.0 * m).mean()

    # overflow path (never hit on sane data): extra launches that
    # full-scan the remaining flagged points, FBC per launch.
    while extra:
        todo = {}
        maps2, order, chunks = [], [], {}
        for b, rest in extra.items():
            f_pad = np.zeros(FBC, np.int64)
            nf = min(len(rest), FBC)
            f_pad[:nf] = rest[:nf]
            maps2.append({
                "hla": in_maps[b]["hla"],
                "hlo": in_maps[b]["hlo"],
                "hlf": np.ascontiguousarray(
                    np.asarray(in_maps[b]["hla"])[:, f_pad]),
            })
            order.append(b)
            chunks[b] = (rest[:nf], nf)
            if len(rest) > nf:
                todo[b] = rest[nf:]
        res2 = run_bass_kernel_spmd(nc, maps2,
                                    core_ids=list(range(len(maps2))))
        for i, b in enumerate(order):
            ia, aS64, fidx, _ = post[b]
            gv2 = np.asarray(res2.results[i]["out"], np.float64)
            fmin = _fb_mins(gv2)
            rest, nf = chunks[b]
            gv = np.asarray(res.results[b]["out"], np.float64)
            mw = gv[:, :KT].T.reshape(K)
            delta = (fmin[:nf] - mw[rest]) * 2.0 / K
            loss1[b] += delta.sum()
        extra = todo

    loss = float((loss1 * weights.astype(np.float64)).mean())
    return np.array(loss, dtype=np.float32)


if __name__ == "__main__":
    rng = np.random.default_rng(0)
    a = rng.standard_normal((B, K, 3), dtype=np.float32)
    o = rng.standard_normal((B, K, 3), dtype=np.float32)
    w = np.ones((B,), dtype=np.float32)
    print(kernel(a, o, w))
